# revision 20
# baseline (speedup 1.0000x reference)
"""Trainium2 Bass kernel for a 2-layer GAT model (GATConv -> ELU -> GATConv -> ELU
-> mean readout -> linear).

Strategy (8 NeuronCores, SPMD, uniform program):
  - Nodes are sharded by dst across the 8 cores; blocks are FIXED 128-node
    ranges (NB = ceil(NSH/128) per core), so every per-block address is a
    compile-time constant and the same program runs on all cores; only table
    CONTENTS differ per core.
  - Per layer a projection table hext[G, 272] bf16 holds, per node row,
    [8 x (32 h-channels | const 1.0) | a_src.h x 8].
    The const-1 slots make the softmax denominator fall out of the same
    aggregation matmul as the numerator.
  - Layer 1: every core computes the FULL table locally from the full x
    (no collective). Layer 2: each core computes its own [SR, 384] slice and
    one AllGather replicates it.
  - Edge pass: per block, K_CH[b] single-index-per-partition indirect DMAs
    (INDIRECT1D) fetch the source rows; the per-block chunk count is the max
    over cores so the program stays uniform. One-hot selectors (tensor_scalar is_equal vs an iota row) turn the
    scatter-sum into PSUM matmuls; scores exp(leaky(as+ad)) are expanded
    head->channel on the Scalar engine; per-edge a_dst comes from small
    S^T @ adb matmuls (S^T via PE transpose), adb being a slice of an
    SBUF-resident per-node ad table filled during the dense phase.
  - The epilogue divides by the aggregated denominator, applies ELU, and
    writes x2 rows with a direct contiguous DMA (layer 1) or accumulates
    the column sum for the mean readout (layer 2).
  - A tiny AllReduce combines per-core column sums; every core finishes the
    linear head redundantly and writes the [1] output.

All graph-dependent tables (gather indices, one-hot slot ids) are built
host-side in numpy; all model FLOPs run on the Trainium cores.
"""

import sys

import numpy as np

for _p in ("/opt/trn_rl_repo",):
    if _p not in sys.path:
        sys.path.insert(0, _p)

from concourse import bass, mybir, tile  # noqa: E402
from concourse.bass import IndirectOffsetOnAxis  # noqa: E402
from concourse.bass_utils import run_bass_kernel_spmd  # noqa: E402
from concourse.masks import make_identity  # noqa: E402

F32 = mybir.dt.float32
BF16 = mybir.dt.bfloat16
I32 = mybir.dt.int32
NP_BF16 = mybir.dt.np(BF16)

N_CORES = 8
NEG_SLOPE = 0.2
RW = 272           # table row width (elements): 8*(32+1) + 8

LEGALIZE_WAITS = True  # sim_test disables: CoreSim's race detector can't track
                       # the synthetic EventSemaphore waits (HW/walrus needs them)


# ----------------------------------------------------------------------------
# Host-side graph preprocessing
# ----------------------------------------------------------------------------
def _preprocess(edge_index: np.ndarray, n_nodes: int, n_cores: int):
    """Partition edges by dst shard; fixed 128-node blocks; per-block chunks."""
    src = np.asarray(edge_index[0], dtype=np.int64)
    dst = np.asarray(edge_index[1], dtype=np.int64)
    nsh = (n_nodes + n_cores - 1) // n_cores
    nb = (nsh + 127) // 128
    sr = nb * 128
    g = n_cores * sr

    owner = np.minimum(dst // nsh, n_cores - 1)
    src_owner = np.minimum(src // nsh, n_cores - 1)
    src_grow = src_owner * sr + (src - src_owner * nsh)

    per_block = [[None] * nb for _ in range(n_cores)]
    ecnt = np.zeros((n_cores, nb), dtype=np.int64)
    for k in range(n_cores):
        lo = k * nsh
        m = owner == k
        ed = dst[m] - lo
        es = src_grow[m]
        blk = ed // 128
        for b in range(nb):
            bm = blk == b
            per_block[k][b] = (es[bm], (ed[bm] - b * 128).astype(np.int64))
            ecnt[k, b] = bm.sum()

    # per-block chunk count: max over cores (program stays uniform)
    kb = ((ecnt.max(axis=0) + 127) // 128).astype(np.int64)
    kb = np.maximum(kb, 1)
    c0 = np.concatenate([[0], np.cumsum(kb)])  # chunk offset per block
    ct = int(c0[-1])

    idx32 = np.zeros((n_cores, 128, ct), dtype=np.int32)
    drel = np.full((n_cores, 128, ct), -1.0, dtype=np.float32)
    for k in range(n_cores):
        for b in range(nb):
            bs, bd = per_block[k][b]
            n = len(bs)
            npad = int(kb[b]) * 128
            iv = np.zeros(npad, dtype=np.int32)
            iv[:n] = bs
            rv = np.full(npad, -1.0, dtype=np.float32)
            rv[:n] = bd.astype(np.float32)
            # position i -> partition i%128, chunk i//128
            idx32[k, :, c0[b]:c0[b + 1]] = iv.reshape(int(kb[b]), 128).T
            drel[k, :, c0[b]:c0[b + 1]] = rv.reshape(int(kb[b]), 128).T

    nv_last = nsh - (nb - 1) * 128
    return dict(SR=sr, G=g, NB=nb, NSH=nsh, KB=[int(x) for x in kb],
                C0=[int(x) for x in c0], CT=ct, NV_LAST=nv_last,
                idx32=idx32, drel=drel)


def _legalize_waits(nc, cap=1):
    """Split multi-wait instructions: the TRN2 engine-instruction encodings hold
    only a limited number of sync-wait commands (walrus: "Too many sync wait
    commands"). Move excess waits onto standalone sequencer EventSemaphore
    instructions inserted just before, on the same engine queue."""
    for bb in nc.main_func.blocks:
        out = []
        n_split = 0
        for ins in bb.instructions:
            si = ins.sync_info
            waits = list(si.on_wait) if si and si.on_wait else []
            if len(waits) <= cap:
                out.append(ins)
                continue
            movable = [
                w for w in waits
                if w.sync_type == "semaphore" and w.wait_mode == "sem-ge-imm"
            ]
            keep = [w for w in waits if w not in movable]
            n_move = min(len(movable), len(waits) - cap)
            for wt in movable[:n_move]:
                ev = mybir.InstEventSemaphore(
                    name=f"{ins.name}-w{n_split}", ins=[], outs=[]
                )
                n_split += 1
                ev.engine = ins.engine
                ev.sync_info = mybir.SyncInfo(on_wait=[wt], on_update=[])
                out.append(ev)
            keep.extend(movable[n_move:])
            ins.sync_info = mybir.SyncInfo(
                on_wait=keep, on_update=list(si.on_update) if si.on_update else []
            )
            out.append(ins)
        bb.instructions = out


# ----------------------------------------------------------------------------
# Bass program
# ----------------------------------------------------------------------------
def _build_program(cfg):
    SR, G, NB = cfg["SR"], cfg["G"], cfg["NB"]
    KB, C0, CT = cfg["KB"], cfg["C0"], cfg["CT"]
    KMAX = max(KB)
    NV_LAST = cfg["NV_LAST"]
    F = cfg["F"]            # input features (128)
    D = cfg["D"]            # hidden = heads*chan (256)
    H = cfg["H"]            # heads (8)
    CH = D // H             # channels per head (32)
    PW = D + H              # packed agg width: 8 x (32 | 1) = 264
    EW = PW + 2 * H         # dense psum width: PW + as + ad = 280
    n_tiles_g = G // 128
    n_tiles = SR // 128
    kd = max(1, D // 128)   # K-tiles for layer-2 dense
    use_bias = cfg["use_bias"]

    nc = bass.Bass()

    x1Tf_p = nc.declare_dram_parameter("x1Tf", [F, G], BF16, isOutput=False)
    x1To_p = nc.declare_dram_parameter("x1To", [F, SR], BF16, isOutput=False)
    idx_p = nc.declare_dram_parameter("idx32", [128, CT], I32, isOutput=False)
    drel_p = nc.declare_dram_parameter("dst_rel", [128, CT], F32, isOutput=False)
    w1e_p = nc.declare_dram_parameter("W1e", [F, EW], BF16, isOutput=False)
    w2e_p = nc.declare_dram_parameter("W2e", [D, EW], BF16, isOutput=False)
    w1ad_p = nc.declare_dram_parameter("W1ad", [F, H], BF16, isOutput=False)
    iota_p = nc.declare_dram_parameter("iota_row", [128, 128], BF16, isOutput=False)
    olast_p = nc.declare_dram_parameter("ones_last", [128, 1], BF16, isOutput=False)
    lwg_p = nc.declare_dram_parameter("linw_g", [1, D], F32, isOutput=False)
    lwuw_p = nc.declare_dram_parameter("linw_uw", [1, 2], F32, isOutput=False)
    uw_p = nc.declare_dram_parameter("uw", [1, 2], F32, isOutput=False)
    lb_p = nc.declare_dram_parameter("lin_b", [1, 1], F32, isOutput=False)
    if use_bias:
        b1_p = nc.declare_dram_parameter("bias1r", [128, D], F32, isOutput=False)
        b2_p = nc.declare_dram_parameter("bias2r", [128, D], F32, isOutput=False)
    out_p = nc.declare_dram_parameter("out", [1, 1], F32, isOutput=True)

    hext1 = nc.dram_tensor("hext1", [G, RW], BF16)
    hext2_own = nc.dram_tensor("hext2_own", [SR, RW], BF16)
    hext2_full = nc.dram_tensor("hext2_full", [G, RW], BF16, addr_space="Shared")
    x2_dram = nc.dram_tensor("x2", [SR, D], BF16)
    cs_in = nc.dram_tensor("cs_in", [1, D], F32)
    cs_out = nc.dram_tensor("cs_out", [1, D], F32, addr_space="Shared")

    rg = [list(range(N_CORES))]

    with tile.TileContext(nc) as tc:
        with (
            tc.tile_pool(name="const", bufs=1) as cp,
            tc.tile_pool(name="dstr", bufs=4) as dstr,
            tc.tile_pool(name="dstg", bufs=3) as dstgp,
            tc.tile_pool(name="gblk", bufs=4) as gp,
            tc.tile_pool(name="sS", bufs=3) as sp_,
            tc.tile_pool(name="sc", bufs=3) as scp,
            tc.tile_pool(name="sce", bufs=3) as sep,
            tc.tile_pool(name="wb", bufs=3) as wbp,
            tc.tile_pool(name="ep", bufs=3) as epp,
            tc.tile_pool(name="x2s", bufs=3) as x2p,
            tc.tile_pool(name="fin", bufs=1) as fp_,
            tc.tile_pool(name="psDen", bufs=2, space="PSUM") as psDen,
            tc.tile_pool(name="psA", bufs=2, space="PSUM") as psA,
            tc.tile_pool(name="psT", bufs=1, space="PSUM") as psT,
            tc.tile_pool(name="psAD", bufs=2, space="PSUM") as psAD,
            tc.tile_pool(name="psC", bufs=1, space="PSUM") as psC,
        ):
            # ---- constants -------------------------------------------------
            idx_sb = cp.tile([128, CT], I32, tag="idx32")
            nc.sync.dma_start(out=idx_sb[:], in_=idx_p[:])
            drel_sb = cp.tile([128, CT], F32, tag="drel")
            nc.sync.dma_start(out=drel_sb[:], in_=drel_p[:])
            w1e_sb = cp.tile([F, EW], BF16, tag="w1e")
            nc.sync.dma_start(out=w1e_sb[:], in_=w1e_p[:])
            w1ad_sb = cp.tile([F, H], BF16, tag="w1ad")
            nc.sync.dma_start(out=w1ad_sb[:], in_=w1ad_p[:])
            w2e_sb = []
            for q in range(kd):
                wt = cp.tile([128, EW], BF16, tag=f"w2e{q}")
                nc.sync.dma_start(out=wt[:], in_=w2e_p[q * 128:(q + 1) * 128, :])
                w2e_sb.append(wt)
            iota_sb = cp.tile([128, 128], BF16, tag="iota")
            nc.sync.dma_start(out=iota_sb[:], in_=iota_p[:])
            ident_sb = cp.tile([128, 128], BF16, tag="ident")
            make_identity(nc, ident_sb[:])
            ones_sb = cp.tile([128, 1], BF16, tag="ones")
            nc.vector.memset(ones_sb[:], 1.0)
            ones_last_sb = cp.tile([128, 1], BF16, tag="ones_last")
            nc.sync.dma_start(out=ones_last_sb[:], in_=olast_p[:])
            ones8_sb = cp.tile([128, H], BF16, tag="ones8")
            nc.vector.memset(ones8_sb[:], 1.0)
            lwg_sb = cp.tile([1, D], F32, tag="lwg")
            nc.sync.dma_start(out=lwg_sb[:], in_=lwg_p[:])
            lwuw_sb = cp.tile([1, 2], F32, tag="lwuw")
            nc.sync.dma_start(out=lwuw_sb[:], in_=lwuw_p[:])
            uw_sb = cp.tile([1, 2], F32, tag="uw")
            nc.sync.dma_start(out=uw_sb[:], in_=uw_p[:])
            lb_sb = cp.tile([1, 1], F32, tag="lb")
            nc.sync.dma_start(out=lb_sb[:], in_=lb_p[:])
            if use_bias:
                b1_sb = cp.tile([128, D], F32, tag="b1")
                nc.sync.dma_start(out=b1_sb[:], in_=b1_p[:])
                b2_sb = cp.tile([128, D], F32, tag="b2")
                nc.sync.dma_start(out=b2_sb[:], in_=b2_p[:])
            ad1_t = cp.tile([128, NB * H], BF16, tag="ad1")
            ad2_t = cp.tile([128, NB * H], BF16, tag="ad2")
            ad_sb = [ad1_t, ad2_t]

            # zero x2 padding rows (read by the transpose, never written)
            zpad = SR - cfg["NSH"]
            if zpad > 0:
                zt = cp.tile([128, D], BF16, tag="zpad")
                nc.vector.memset(zt[:], 0.0)
                nc.sync.dma_start(
                    out=x2_dram[cfg["NSH"]: cfg["NSH"] + zpad, :], in_=zt[0:zpad, :]
                )

            csum_ps = psC.tile([1, D], F32, tag="cs")

            def dense_tile(lhsT_tiles, dst_rows, dst_dram, ad_tile_idx, layer):
                """One 128-row dense tile: matmul -> pack -> DMA table rows."""
                ps = psDen.tile([128, EW], F32, tag="psd")
                for q, lt in enumerate(lhsT_tiles):
                    nc.tensor.matmul(
                        out=ps[:],
                        lhsT=lt,
                        rhs=w1e_sb[:] if layer == 0 else w2e_sb[q][:],
                        start=(q == 0), stop=(q == len(lhsT_tiles) - 1),
                    )
                stg = dstgp.tile([128, EW], BF16, tag="stg")
                nc.scalar.activation(
                    out=stg[:], in_=ps[:],
                    func=mybir.ActivationFunctionType.Copy,
                )
                # const-1 slots for the denominator aggregation
                nc.vector.tensor_copy(
                    out=stg[:, 0:PW].rearrange(
                        "p (h c) -> p h c", c=CH + 1)[:, :, CH],
                    in_=ones8_sb[:],
                )
                nc.sync.dma_start(
                    out=dst_dram[dst_rows[0]:dst_rows[1], 0:PW + H],
                    in_=stg[:, 0:PW + H],
                )
                if ad_tile_idx is not None:
                    t = ad_tile_idx
                    nc.vector.tensor_copy(
                        out=ad_sb[layer][:, t * H:(t + 1) * H],
                        in_=stg[:, PW + H:EW],
                    )

            # ---- Phase A: layer-1 dense (full table, redundant) ------------
            for t in range(n_tiles_g):
                xt = dstr.tile([128, 128], BF16, tag="xt")
                nc.sync.dma_start(out=xt[:], in_=x1Tf_p[:, t * 128:(t + 1) * 128])
                dense_tile([xt[:]], (t * 128, (t + 1) * 128), hext1, None, 0)
            # ad pass over own shard
            for t in range(n_tiles):
                xt = dstr.tile([128, 128], BF16, tag="xt")
                nc.sync.dma_start(out=xt[:], in_=x1To_p[:, t * 128:(t + 1) * 128])
                pa = psAD.tile([128, KMAX * H], F32, tag="ps_ad")
                nc.tensor.matmul(out=pa[:, 0:H], lhsT=xt[:], rhs=w1ad_sb[:],
                                 start=True, stop=True)
                nc.vector.tensor_copy(
                    out=ad_sb[0][:, t * H:(t + 1) * H], in_=pa[:, 0:H]
                )

            # ---- edge pass (shared for both layers) ------------------------
            def edge_pass(layer, table_ap):
                for b in range(NB):
                    nv = 128 if b < NB - 1 else NV_LAST
                    KBB = KB[b]
                    cb0 = C0[b]
                    gblk = gp.tile([128, KMAX * RW], BF16, tag="gblk")
                    for kk in range(KBB):
                        nc.gpsimd.indirect_dma_start(
                            out=gblk[:, kk * RW:(kk + 1) * RW],
                            out_offset=None,
                            in_=table_ap[:],
                            in_offset=IndirectOffsetOnAxis(
                                ap=idx_sb[:, cb0 + kk:cb0 + kk + 1], axis=0
                            ),
                        )
                    adb = ad_sb[layer][:, b * H:(b + 1) * H]
                    # one-hot S per chunk (edges x slots)
                    s_all = sp_.tile([128, KMAX * 128], BF16, tag="s_all")
                    st_all = sp_.tile([128, KMAX * 128], BF16, tag="st_all")
                    ps_ad = psAD.tile([128, KMAX * H], F32, tag="ps_ad")
                    for kk in range(KBB):
                        nc.vector.tensor_scalar(
                            out=s_all[:, kk * 128:(kk + 1) * 128],
                            in0=iota_sb[:],
                            scalar1=drel_sb[:, cb0 + kk: cb0 + kk + 1],
                            scalar2=None,
                            op0=mybir.AluOpType.is_equal,
                        )
                    for q4 in range(0, KBB, 4):
                        nq = min(4, KBB - q4)
                        pst = psT.tile([128, 4 * 128], BF16, tag="pst")
                        for j in range(nq):
                            kk = q4 + j
                            nc.tensor.transpose(
                                out=pst[:, j * 128:(j + 1) * 128],
                                in_=s_all[:, kk * 128:(kk + 1) * 128],
                                identity=ident_sb[:],
                            )
                        nc.vector.tensor_copy(
                            out=st_all[:, q4 * 128:(q4 + nq) * 128],
                            in_=pst[:, 0:nq * 128],
                        )
                        for j in range(nq):
                            kk = q4 + j
                            nc.tensor.matmul(
                                out=ps_ad[:, kk * H:(kk + 1) * H],
                                lhsT=st_all[:, kk * 128:(kk + 1) * 128],
                                rhs=adb,
                                start=True, stop=True,
                            )
                    # scores: e = leaky(as + ad); sce = exp(e) expanded to 33
                    adc = scp.tile([128, KMAX * H], BF16, tag="adc")
                    nc.vector.tensor_copy(
                        out=adc[:, 0:KBB * H], in_=ps_ad[:, 0:KBB * H])
                    scc = scp.tile([128, KMAX * H], BF16, tag="scc")
                    as_view = gblk[:, 0:KBB * RW].rearrange(
                        "p (g w) -> p g w", w=RW)[:, :, PW:PW + H]
                    nc.vector.tensor_tensor(
                        out=scc[:, 0:KBB * H].rearrange("p (g h) -> p g h", h=H),
                        in0=as_view,
                        in1=adc[:, 0:KBB * H].rearrange("p (g h) -> p g h", h=H),
                        op=mybir.AluOpType.add,
                    )
                    t2 = scp.tile([128, KMAX * H], BF16, tag="t2")
                    nc.vector.tensor_scalar_mul(
                        out=t2[:, 0:KBB * H], in0=scc[:, 0:KBB * H],
                        scalar1=NEG_SLOPE)
                    nc.vector.tensor_tensor(
                        out=scc[:, 0:KBB * H], in0=scc[:, 0:KBB * H],
                        in1=t2[:, 0:KBB * H], op=mybir.AluOpType.max
                    )
                    sce = sep.tile([128, KMAX * PW], BF16, tag="sce")
                    nc.scalar.activation(
                        out=sce[:, 0:KBB * PW],
                        in_=scc[:, 0:KBB * H].rearrange("p (g h) -> p g h", h=H)
                        .unsqueeze(-1).to_broadcast([128, KBB, H, CH + 1]),
                        func=mybir.ActivationFunctionType.Exp,
                    )
                    wblk = wbp.tile([128, KMAX * PW], BF16, tag="wblk")
                    nc.vector.tensor_tensor(
                        out=wblk[:, 0:KBB * PW].rearrange(
                            "p (g w) -> p g w", w=PW),
                        in0=gblk[:, 0:KBB * RW].rearrange(
                            "p (g w) -> p g w", w=RW)[:, :, 0:PW],
                        in1=sce[:, 0:KBB * PW].rearrange(
                            "p (g w) -> p g w", w=PW),
                        op=mybir.AluOpType.mult,
                    )
                    # aggregation (numerator + denominator in one matmul chain)
                    ps_o = psA.tile([128, PW], F32, tag="ps_o")
                    for kk in range(KBB):
                        nc.tensor.matmul(
                            out=ps_o[:],
                            lhsT=s_all[:, kk * 128:(kk + 1) * 128],
                            rhs=wblk[:, kk * PW:(kk + 1) * PW],
                            start=(kk == 0), stop=(kk == KBB - 1),
                        )
                    # ---- epilogue ----------------------------------------
                    rd = epp.tile([128, H], F32, tag="rd")
                    nc.vector.tensor_scalar_add(
                        out=rd[:],
                        in0=ps_o[:].rearrange("p (h c) -> p h c", c=CH + 1)[:, :, CH],
                        scalar1=1e-16,
                    )
                    nc.vector.reciprocal(out=rd[:], in_=rd[:])
                    rde = epp.tile([128, D], F32, tag="rde")
                    nc.scalar.activation(
                        out=rde[:],
                        in_=rd[:].unsqueeze(-1).to_broadcast([128, H, CH]),
                        func=mybir.ActivationFunctionType.Copy,
                    )
                    st = epp.tile([128, D], F32, tag="st")
                    nc.vector.tensor_tensor(
                        out=st[:].rearrange("p (h c) -> p h c", c=CH),
                        in0=ps_o[:].rearrange("p (h c) -> p h c", c=CH + 1)[:, :, 0:CH],
                        in1=rde[:].rearrange("p (h c) -> p h c", c=CH),
                        op=mybir.AluOpType.mult,
                    )
                    if use_bias:
                        bsb = b1_sb if layer == 0 else b2_sb
                        nc.vector.tensor_tensor(
                            out=st[:], in0=st[:], in1=bsb[:], op=mybir.AluOpType.add
                        )
                    tm = epp.tile([128, D], F32, tag="tm")
                    nc.vector.tensor_scalar_min(out=tm[:], in0=st[:], scalar1=0.0)
                    nc.scalar.activation(
                        out=tm[:], in_=tm[:], func=mybir.ActivationFunctionType.Exp
                    )
                    nc.vector.tensor_scalar(
                        out=st[:], in0=st[:],
                        scalar1=0.0, scalar2=-1.0,
                        op0=mybir.AluOpType.max, op1=mybir.AluOpType.add,
                    )
                    xs = x2p.tile([128, D], BF16, tag="xs")
                    nc.vector.tensor_tensor(
                        out=xs[:], in0=st[:], in1=tm[:], op=mybir.AluOpType.add
                    )
                    if layer == 0:
                        nc.sync.dma_start(
                            out=x2_dram[b * 128: b * 128 + nv, :], in_=xs[0:nv, :]
                        )
                    else:
                        nc.tensor.matmul(
                            out=csum_ps[:],
                            lhsT=ones_sb[:] if b < NB - 1 else ones_last_sb[:],
                            rhs=xs[:],
                            start=(b == 0), stop=(b == NB - 1),
                        )

            # ---- Phase B: layer-1 edge pass --------------------------------
            edge_pass(0, hext1)

            # ---- Phase C: layer-2 dense (own shard) + AllGather ------------
            x2T_sb = []
            for q in range(kd):
                xt = cp.tile([128, SR], BF16, tag=f"x2T{q}")
                nc.sync.dma_start_transpose(
                    out=xt[:], in_=x2_dram[:, q * 128:(q + 1) * 128]
                )
                x2T_sb.append(xt)
            for t in range(n_tiles):
                dense_tile(
                    [x2T_sb[q][:, t * 128:(t + 1) * 128] for q in range(kd)],
                    (t * 128, (t + 1) * 128), hext2_own, t, 1,
                )
            nc.gpsimd.collective_compute(
                "AllGather",
                mybir.AluOpType.bypass,
                ins=[hext2_own[:]],
                outs=[hext2_full[:]],
                replica_groups=rg,
            )

            # ---- Phase D: layer-2 edge pass --------------------------------
            edge_pass(1, hext2_full)

            # ---- readout ---------------------------------------------------
            cs_sb = fp_.tile([1, D], F32, tag="cs_sb")
            nc.vector.tensor_copy(out=cs_sb[:], in_=csum_ps[:])
            nc.sync.dma_start(out=cs_in[:], in_=cs_sb[:])
            nc.gpsimd.collective_compute(
                "AllReduce",
                mybir.AluOpType.add,
                ins=[cs_in[:]],
                outs=[cs_out[:]],
                replica_groups=rg,
            )
            cs2 = fp_.tile([1, D], F32, tag="cs2")
            nc.sync.dma_start(out=cs2[:], in_=cs_out[:])
            tg = fp_.tile([1, D], F32, tag="tg")
            acc1 = fp_.tile([1, 1], F32, tag="acc1")
            nc.vector.tensor_tensor(
                out=tg[:], in0=cs2[:], in1=lwg_sb[:], op=mybir.AluOpType.mult
            )
            nc.vector.tensor_reduce(
                out=acc1[:], in_=tg[:], axis=mybir.AxisListType.X,
                op=mybir.AluOpType.add,
            )
            t2f = fp_.tile([1, 2], F32, tag="t2f")
            acc2 = fp_.tile([1, 1], F32, tag="acc2")
            nc.vector.tensor_tensor(
                out=t2f[:], in0=uw_sb[:], in1=lwuw_sb[:], op=mybir.AluOpType.mult
            )
            nc.vector.tensor_reduce(
                out=acc2[:], in_=t2f[:], axis=mybir.AxisListType.X,
                op=mybir.AluOpType.add,
            )
            nc.vector.tensor_tensor(
                out=acc1[:], in0=acc1[:], in1=acc2[:], op=mybir.AluOpType.add
            )
            nc.vector.tensor_tensor(
                out=acc1[:], in0=acc1[:], in1=lb_sb[:], op=mybir.AluOpType.add
            )
            nc.sync.dma_start(out=out_p[:], in_=acc1[:])

    if LEGALIZE_WAITS:
        _legalize_waits(nc)
    return nc


# ----------------------------------------------------------------------------
# Host-side input assembly
# ----------------------------------------------------------------------------
def _att_matrix(att: np.ndarray) -> np.ndarray:
    Hh, Cc = att.shape
    A = np.zeros((Hh * Cc, Hh), dtype=np.float64)
    for h in range(Hh):
        A[h * Cc:(h + 1) * Cc, h] = att[h]
    return A


def _pack_we(W, a_s, a_d, H, CH):
    """Packed dense weights: [8 x (32 W-cols | 0) | W@As | W@Ad] (K x 280)."""
    K = W.shape[0]
    Ms = W @ _att_matrix(a_s)   # [K, H]
    Md = W @ _att_matrix(a_d)   # [K, H]
    PW = H * (CH + 1)
    out = np.zeros((K, PW + 2 * H), dtype=np.float64)
    for h in range(H):
        out[:, h * (CH + 1): h * (CH + 1) + CH] = W[:, h * CH:(h + 1) * CH]
    out[:, PW:PW + H] = Ms
    out[:, PW + H:PW + 2 * H] = Md
    return out


def _make_inputs(prep, cfg, x, u, w, W1, as1, ad1, b1, W2, as2, ad2, b2,
                 lin_w, lin_b):
    SR, NSH, G = prep["SR"], prep["NSH"], prep["G"]
    F, D, H = cfg["F"], cfg["D"], cfg["H"]
    CH = D // H
    n_nodes = x.shape[0]

    W1e = _pack_we(W1, as1, ad1, H, CH).astype(NP_BF16)
    W2e = _pack_we(W2, as2, ad2, H, CH).astype(NP_BF16)
    W1ad = (W1 @ _att_matrix(ad1)).astype(NP_BF16)
    iota_row = np.tile(np.arange(128, dtype=np.float32), (128, 1)).astype(NP_BF16)
    ones_last = (np.arange(128) < prep["NV_LAST"]).astype(np.float32).reshape(
        128, 1).astype(NP_BF16)
    linw_g = (lin_w[0, :D] / float(n_nodes)).astype(np.float32).reshape(1, D)
    linw_uw = lin_w[0, D:D + 2].astype(np.float32).reshape(1, 2)
    uwv = np.array([[float(u), float(w)]], dtype=np.float32)
    lbv = np.asarray(lin_b, dtype=np.float32).reshape(1, 1)

    # full padded x^T, identical on every core
    xf = np.zeros((G, F), dtype=np.float32)
    for k in range(N_CORES):
        lo = k * NSH
        hi = min(lo + NSH, n_nodes)
        xf[k * SR: k * SR + hi - lo] = x[lo:hi]
    x1Tf = np.ascontiguousarray(xf.T).astype(NP_BF16)

    in_maps = []
    for k in range(N_CORES):
        m = {
            "x1Tf": x1Tf,
            "x1To": np.ascontiguousarray(x1Tf[:, k * SR:(k + 1) * SR]),
            "idx32": prep["idx32"][k],
            "dst_rel": prep["drel"][k],
            "W1e": W1e,
            "W2e": W2e,
            "W1ad": W1ad,
            "iota_row": iota_row,
            "ones_last": ones_last,
            "linw_g": linw_g,
            "linw_uw": linw_uw,
            "uw": uwv,
            "lin_b": lbv,
        }
        if cfg["use_bias"]:
            m["bias1r"] = np.tile(b1.astype(np.float32), (128, 1))
            m["bias2r"] = np.tile(b2.astype(np.float32), (128, 1))
        in_maps.append(m)
    return in_maps


def build_all(x, edge_index, u, w, W1, att_src1, att_dst1, bias1,
              W2, att_src2, att_dst2, bias2, lin_w, lin_b, **_ignored):
    n_nodes, F = x.shape
    H, Cc = att_src1.shape
    D = H * Cc
    use_bias = bool(np.any(bias1) or np.any(bias2))
    prep = _preprocess(np.asarray(edge_index), n_nodes, N_CORES)
    cfg = dict(prep)
    cfg.pop("idx32"), cfg.pop("drel")
    cfg.update(F=F, D=D, H=H, use_bias=use_bias)
    nc = _build_program(cfg)
    in_maps = _make_inputs(
        prep, cfg, np.asarray(x, np.float32), u, w,
        np.asarray(W1, np.float64), np.asarray(att_src1, np.float64),
        np.asarray(att_dst1, np.float64), np.asarray(bias1, np.float64),
        np.asarray(W2, np.float64), np.asarray(att_src2, np.float64),
        np.asarray(att_dst2, np.float64), np.asarray(bias2, np.float64),
        np.asarray(lin_w, np.float64), np.asarray(lin_b, np.float64),
    )
    return nc, in_maps


def kernel(**inputs) -> np.ndarray:
    nc, in_maps = build_all(
        inputs["x"], inputs["edge_index"], inputs["u"], inputs["w"],
        inputs["W1"], inputs["att_src1"], inputs["att_dst1"], inputs["bias1"],
        inputs["W2"], inputs["att_src2"], inputs["att_dst2"], inputs["bias2"],
        inputs["lin_w"], inputs["lin_b"],
    )
    res = run_bass_kernel_spmd(nc, in_maps, core_ids=list(range(N_CORES)))
    return res.results[0]["out"].reshape(1).astype(np.float32)
